# revision 6
# baseline (speedup 1.0000x reference)
"""HGNN (2-layer hetero GraphSAGE + 8 heads) on 8 trn2 NeuronCores.

Sharding: dst-node interleaved (core = v % 8, local = v // 8). Each layer is
one SPMD NEFF launch; the host performs the inter-layer halo exchange by
concatenating per-core outputs into fresh gather tables (indices are
pre-translated into the concatenated layout).

Device-side per layer, per core:
  - For each 512-dst-column PSUM group, edges (sorted by dst) are cut into
    128-edge windows on a column grid that is uniform across cores
    (min-over-cores advance), so a single program serves all 8 cores.
  - Per window: one indirect DMA gathers the 128 source rows [128, 128];
    a selection matrix sel[e, j] = (rel_dst[e] == j) * invcnt[e] is built
    with two batched DVE ops; PE accumulates g.T @ sel into the PSUM group,
    yielding the transposed scatter-mean m^T [128 feat, 512 dst] directly.
  - Dense stage: nb^T = Wl_bb.T @ m_bb^T + Wl_sb.T @ m_sb^T + Wr.T @ x^T,
    then bias + leaky-relu fused on the scalar engine. Head (layer 2) is one
    more matmul with Wh^T producing y^T [8, dst].
"""
import os
import time
import numpy as np

import concourse.bass as bass
import concourse.bacc as bacc
import concourse.mybir as mybir
import concourse.tile as tile
from concourse.bass_utils import run_bass_kernel_spmd

P = 128
D = 128
NCORES = 8
GROUP = 512       # psum columns per accumulation group
S = 128           # max dst-column span per 128-edge window
BUCK = 25000      # src rows per int16 gather bucket
NB, NS = 100000, 50000
NLB, NLS = NB // NCORES, NS // NCORES   # 12500, 6250


# ---------------------------------------------------------------- host prep
def _shard_edges(src, dst, n_dst):
    """Split edges by dst core; per core return (src, dst_local) dst-sorted."""
    core = dst % NCORES
    loc = dst // NCORES
    out = []
    for c in range(NCORES):
        m = core == c
        s, d = src[m], loc[m]
        o = np.argsort(d, kind="stable")
        out.append((s[o].astype(np.int64), d[o].astype(np.int64)))
    return out


def _pack_type(per_core, n_loc, n_src):
    """Bucketed uniform-across-cores window packing for dma_gather.

    Edges are split by src bucket (BUCK rows each, int16-addressable); per
    (group, bucket) windows advance on a column grid uniform across cores.
    Returns (idx16 per bucket: list of [NCORES, 128, cols_b],
             rel [NCORES, P, Wtot], invc [NCORES, P, Wtot],
             groups: per group list of (bucket, k_local, col_off, span),
             gb_meta: per group dict bucket -> (idx_col_base, Nk))."""
    nbuck = (n_src + BUCK - 1) // BUCK
    ngroups = (n_loc + GROUP - 1) // GROUP
    # per (core, bucket): dst-sorted edge arrays + prefix counts
    pcb = [[None] * nbuck for _ in range(NCORES)]
    cumb = [[None] * nbuck for _ in range(NCORES)]
    counts_all = []
    for cc, (s, d) in enumerate(per_core):
        counts_all.append(np.bincount(d, minlength=n_loc))
        for b in range(nbuck):
            m = (s >= b * BUCK) & (s < (b + 1) * BUCK)
            sb_, db_ = s[m], d[m]
            pcb[cc][b] = (sb_ - b * BUCK, db_)
            cnt = np.bincount(db_, minlength=n_loc)
            cumb[cc][b] = np.concatenate([[0], np.cumsum(cnt)])
    invc_dst = [1.0 / np.maximum(c, 1) for c in counts_all]

    groups, gb_meta = [], []
    rel_cols = [[] for _ in range(NCORES)]
    invc_cols = [[] for _ in range(NCORES)]
    idx_flat = [[[] for _ in range(nbuck)] for _ in range(NCORES)]
    idx_base = [0] * nbuck
    for g in range(ngroups):
        c0, c1 = g * GROUP, min((g + 1) * GROUP, n_loc)
        wins, meta = [], {}
        for b in range(nbuck):
            k_local = 0
            c = c0
            while c < c1:
                span = min(S, c1 - c)
                while span > 1:
                    ok = all(cumb[cc][b][c + span] - cumb[cc][b][c] <= P
                             for cc in range(NCORES))
                    if ok:
                        break
                    span -= 1
                for cc in range(NCORES):
                    s_arr, d_arr = pcb[cc][b]
                    a2, b2 = cumb[cc][b][c], cumb[cc][b][c + span]
                    n = b2 - a2
                    assert n <= P
                    icol = np.zeros(P, np.int16)
                    rcol = np.full(P, -1.0, np.float32)
                    vcol = np.zeros(P, np.float32)
                    icol[:n] = s_arr[a2:b2].astype(np.int16)
                    rcol[:n] = (d_arr[a2:b2] - c).astype(np.float32)
                    vcol[:n] = invc_dst[cc][d_arr[a2:b2]].astype(np.float32)
                    idx_flat[cc][b].append(icol)
                    rel_cols[cc].append(rcol)
                    invc_cols[cc].append(vcol)
                wins.append((b, k_local, c - c0, span))
                k_local += 1
                c += span
            if k_local:
                meta[b] = (idx_base[b], k_local * P)
                idx_base[b] += k_local * P
        groups.append(wins)
        gb_meta.append(meta)

    # int16 device layout per bucket: flat i at [i%16, i//16], tiled 8x down
    idx16 = []
    for b in range(nbuck):
        per_core_arr = []
        for cc in range(NCORES):
            flat = np.concatenate(idx_flat[cc][b]) if idx_flat[cc][b] else np.zeros(0, np.int16)
            blk = flat.reshape(-1, 16).T          # [16, cols]
            per_core_arr.append(np.tile(blk, (8, 1)))
        idx16.append(np.stack(per_core_arr).astype(np.int16))
    rel = np.stack([np.stack(cols, 1) for cols in rel_cols]).astype(np.float32)
    invc = np.stack([np.stack(cols, 1) for cols in invc_cols]).astype(np.float32)
    return idx16, rel, invc, groups, gb_meta


# ------------------------------------------------------------- device build
def _build_launch(cfg):
    """Build one layer's SPMD program. cfg keys:
      tabs: {name: nrows} gather tables
      types: list of dicts(name, tab, Wtot, groups, n_loc)
      head: bool — add 8-head output (layer 2)
      out_s: bool — emit s-node output (layer 1)
    """
    nc = bacc.Bacc("TRN2", target_bir_lowering=False, debug=False,
                   num_devices=NCORES)
    f32, i32 = mybir.dt.float32, mybir.dt.int32

    i16 = mybir.dt.int16
    d_tab = {}
    for k, n in cfg["tabs"].items():
        nb_ = (n + BUCK - 1) // BUCK
        d_tab[k] = [nc.dram_tensor(f"{k}_{b}", [min(BUCK, n - b * BUCK), D], f32,
                                   kind="ExternalInput") for b in range(nb_)]
    d_xbT = nc.dram_tensor("xbT", [P, NLB], f32, kind="ExternalInput")
    d_xsT = (nc.dram_tensor("xsT", [P, NLS], f32, kind="ExternalInput")
             if cfg["out_s"] else None)
    # packed weights: Wl_bb | Wl_sb | Wr_b | [Wl_bs | Wr_s] | WhT | iota | biases
    nw = 3 * D + (2 * D if cfg["out_s"] else 0) + (8 if cfg["head"] else 0) + S + 3
    d_w = nc.dram_tensor("wts", [P, nw], f32, kind="ExternalInput")
    d_et = {}
    for t in cfg["types"]:
        W = t["Wtot"]
        d_et[t["name"]] = (
            [nc.dram_tensor(f'idx_{t["name"]}_{b}', [P, max(t["bcols"][b], 16)],
                            i16, kind="ExternalInput")
             for b in range(len(t["bcols"]))],
            nc.dram_tensor(f'rel_{t["name"]}', [P, W], f32, kind="ExternalInput"),
            nc.dram_tensor(f'ivc_{t["name"]}', [P, W], f32, kind="ExternalInput"),
        )
    d_nbT = nc.dram_tensor("nbT", [P, NLB], f32, kind="ExternalOutput")
    d_nsT = (nc.dram_tensor("nsT", [P, NLS], f32, kind="ExternalOutput")
             if cfg["out_s"] else None)
    d_yT = (nc.dram_tensor("yT", [8, NLB], f32, kind="ExternalOutput")
            if cfg["head"] else None)

    types = {t["name"]: t for t in cfg["types"]}

    from contextlib import ExitStack
    with tile.TileContext(nc) as tc, ExitStack() as ctx:
        wpool = ctx.enter_context(tc.tile_pool(name="w", bufs=1))
        gpool = ctx.enter_context(tc.tile_pool(name="g", bufs=5))
        selpool = ctx.enter_context(tc.tile_pool(name="sel", bufs=2))
        mpool = ctx.enter_context(tc.tile_pool(name="m", bufs=3))
        spool = ctx.enter_context(tc.tile_pool(name="s", bufs=3))
        appool = ctx.enter_context(tc.tile_pool(name="ap", bufs=3, space="PSUM"))
        s2pool = ctx.enter_context(tc.tile_pool(name="s2", bufs=2, space="PSUM"))
        hpool = (ctx.enter_context(tc.tile_pool(name="h", bufs=2, space="PSUM"))
                 if cfg["head"] else None)

        t_w = wpool.tile([P, nw], f32)
        nc.sync.dma_start(t_w[:], d_w[:])
        off = 0
        w_Wlbb = t_w[:, off:off + D]; off += D
        w_Wlsb = t_w[:, off:off + D]; off += D
        w_Wrb = t_w[:, off:off + D]; off += D
        if cfg["out_s"]:
            w_Wlbs = t_w[:, off:off + D]; off += D
            w_Wrs = t_w[:, off:off + D]; off += D
        if cfg["head"]:
            w_WhT = t_w[:, off:off + 8]; off += 8
        w_iota = t_w[:, off:off + S]; off += S
        w_bb = t_w[:, off:off + 1]; off += 1
        w_bs = t_w[:, off:off + 1]; off += 1
        w_bh = t_w[:, off:off + 1]; off += 1

        def aggregate(tname, g, wbase):
            """Aggregate one group of `tname` into a PSUM tile."""
            t = types[tname]
            d_idxb, d_rel, d_ivc = d_et[tname]
            wins = t["groups"][g]        # (bucket, k_local, col_off, span)
            meta = t["gb_meta"][g]       # bucket -> (slot_base, Nk)
            Wg = len(wins)
            ncols = max(c + s for (_, _, c, s) in wins)
            t_rel = mpool.tile([P, Wg], f32, tag="rel")
            nc.sync.dma_start(t_rel[:], d_rel[:, wbase:wbase + Wg])
            t_ivc = mpool.tile([P, Wg], f32, tag="ivc")
            nc.sync.dma_start(t_ivc[:], d_ivc[:, wbase:wbase + Wg])
            gtiles = {}
            for b, (sbase, Nk) in sorted(meta.items()):
                t_idx = mpool.tile([P, Nk // 16], mybir.dt.int16, tag="idx")
                nc.sync.dma_start(
                    t_idx[:], d_idxb[b][:, sbase // 16:(sbase + Nk) // 16])
                t_gb = gpool.tile([P, (Nk // P) * D], f32, tag="gb")
                nc.gpsimd.dma_gather(
                    out_ap=t_gb[:].rearrange("p (k d) -> p k d", k=Nk // P),
                    in_ap=d_tab[t["tab"]][b][:], idxs_ap=t_idx[:],
                    num_idxs=Nk, num_idxs_reg=Nk, elem_size=D,
                    single_packet=False)
                gtiles[b] = t_gb
            t_sel = selpool.tile([P, Wg * S], f32, tag="sel")
            sel3 = t_sel[:].rearrange("p (w s) -> p w s", w=Wg)
            nc.vector.tensor_tensor(
                out=sel3, in0=t_rel[:, :, None].to_broadcast([P, Wg, S]),
                in1=w_iota[:, None, :].to_broadcast([P, Wg, S]),
                op=mybir.AluOpType.is_equal)
            nc.vector.tensor_tensor(
                out=sel3, in0=sel3,
                in1=t_ivc[:, :, None].to_broadcast([P, Wg, S]),
                op=mybir.AluOpType.mult)
            t_ps = appool.tile([P, GROUP], f32, space="PSUM", tag="agg")
            for w, (b, k, coff, span) in enumerate(wins):
                nc.tensor.matmul(
                    t_ps[:, coff:coff + span],
                    lhsT=gtiles[b][:, k * D:(k + 1) * D],
                    rhs=t_sel[:, w * S:w * S + span],
                    start=(w == 0), stop=(w == Wg - 1))
            t_m = spool.tile([P, GROUP], f32, tag="mT")
            nc.vector.tensor_copy(out=t_m[:, :ncols], in_=t_ps[:, :ncols])
            return t_m, ncols

        # ---- b-node groups
        ngb = len(types["bb"]["groups"])
        ngs_on_b = len(types["sb"]["groups"])
        wb_bb = 0
        wb_sb = 0
        for g in range(ngb):
            m_bb, ncols = aggregate("bb", g, wb_bb)
            wb_bb += len(types["bb"]["groups"][g])
            has_sb = g < ngs_on_b
            if has_sb:
                m_sb, ncols_sb = aggregate("sb", g, wb_sb)
                wb_sb += len(types["sb"]["groups"][g])
            t_x = spool.tile([P, GROUP], f32, tag="xg")
            nc.sync.dma_start(t_x[:, :ncols],
                              d_xbT[:, g * GROUP:g * GROUP + ncols])
            ps2 = s2pool.tile([P, GROUP], f32, space="PSUM", tag="s2")
            nc.tensor.matmul(ps2[:, :ncols], lhsT=w_Wlbb, rhs=m_bb[:, :ncols],
                             start=True, stop=False)
            if has_sb:
                nc.tensor.matmul(ps2[:, :ncols_sb], lhsT=w_Wlsb,
                                 rhs=m_sb[:, :ncols_sb],
                                 start=False, stop=False)
            nc.tensor.matmul(ps2[:, :ncols], lhsT=w_Wrb, rhs=t_x[:, :ncols],
                             start=False, stop=True)
            t_o = spool.tile([P, GROUP], f32, tag="ob")
            nc.scalar.activation(out=t_o[:, :ncols], in_=ps2[:, :ncols],
                                 func=mybir.ActivationFunctionType.Lrelu,
                                 bias=w_bb, alpha=0.01)
            nc.sync.dma_start(d_nbT[:, g * GROUP:g * GROUP + ncols],
                              t_o[:, :ncols])
            if cfg["head"]:
                ps3 = hpool.tile([8, GROUP], f32, space="PSUM", tag="hd")
                nc.tensor.matmul(ps3[:, :ncols], lhsT=w_WhT,
                                 rhs=t_o[:, :ncols], start=True, stop=True)
                t_y = spool.tile([8, GROUP], f32, tag="yt")
                nc.vector.tensor_scalar_add(t_y[:, :ncols], ps3[:, :ncols],
                                            w_bh[:8])
                nc.sync.dma_start(d_yT[:, g * GROUP:g * GROUP + ncols],
                                  t_y[:, :ncols])

        # ---- s-node groups (layer 1 only)
        if cfg["out_s"]:
            wb_bs = 0
            for g in range(len(types["bs"]["groups"])):
                m_bs, ncols = aggregate("bs", g, wb_bs)
                wb_bs += len(types["bs"]["groups"][g])
                t_x = spool.tile([P, GROUP], f32, tag="xg")
                nc.sync.dma_start(t_x[:, :ncols],
                                  d_xsT[:, g * GROUP:g * GROUP + ncols])
                ps2 = s2pool.tile([P, GROUP], f32, space="PSUM", tag="s2")
                nc.tensor.matmul(ps2[:, :ncols], lhsT=w_Wlbs,
                                 rhs=m_bs[:, :ncols], start=True, stop=False)
                nc.tensor.matmul(ps2[:, :ncols], lhsT=w_Wrs,
                                 rhs=t_x[:, :ncols], start=False, stop=True)
                t_o = spool.tile([P, GROUP], f32, tag="ob")
                nc.scalar.activation(out=t_o[:, :ncols], in_=ps2[:, :ncols],
                                     func=mybir.ActivationFunctionType.Lrelu,
                                     bias=w_bs, alpha=0.01)
                nc.sync.dma_start(d_nsT[:, g * GROUP:g * GROUP + ncols],
                                  t_o[:, :ncols])

    nc.compile()
    return nc


def _pack_weights(cfg, Wlbb, Wlsb, Wrb, bb, bs_bias=None, Wlbs=None, Wrs=None,
                  WhT=None, bh0=None):
    nw = 3 * D + (2 * D if cfg["out_s"] else 0) + (8 if cfg["head"] else 0) + S + 3
    w = np.zeros((P, nw), np.float32)
    off = 0
    for M in [Wlbb, Wlsb, Wrb]:
        w[:, off:off + D] = M; off += D
    if cfg["out_s"]:
        w[:, off:off + D] = Wlbs; off += D
        w[:, off:off + D] = Wrs; off += D
    if cfg["head"]:
        w[:, off:off + 8] = WhT; off += 8
    w[:, off:off + S] = np.arange(S, dtype=np.float32)[None, :]; off += S
    w[:, off] = bb; off += 1
    if bs_bias is not None:
        w[:, off] = bs_bias
    off += 1
    if bh0 is not None:
        w[:8, off] = bh0
    return w


LAST_HW_NS = None
LAST_EXEC_S = None


def kernel(x_b, x_s, Wl, bl, Wr, Wh, bh, ei_bb, ei_sb, ei_bs):
    x_b = np.asarray(x_b, np.float32); x_s = np.asarray(x_s, np.float32)
    Wl = np.asarray(Wl, np.float32); bl = np.asarray(bl, np.float32)
    Wr = np.asarray(Wr, np.float32); Wh = np.asarray(Wh, np.float32)
    bh = np.asarray(bh, np.float32)
    ei_bb = np.asarray(ei_bb); ei_sb = np.asarray(ei_sb); ei_bs = np.asarray(ei_bs)

    # ---------------- layer 1 prep (original node ids as gather indices)
    pc_bb = _shard_edges(ei_bb[0], ei_bb[1], NB)
    pc_sb = _shard_edges(ei_sb[0], ei_sb[1], NB)   # dst are b-nodes < NS
    pc_bs = _shard_edges(ei_bs[0], ei_bs[1], NS)
    i_bb, r_bb, v_bb, g_bb, m_bb = _pack_type(pc_bb, NLB, NB)
    i_sb, r_sb, v_sb, g_sb, m_sb = _pack_type(pc_sb, NS // NCORES, NS)
    i_bs, r_bs, v_bs, g_bs, m_bs = _pack_type(pc_bs, NLS, NS)

    cfgA = {
        "tabs": {"tab_b": NB, "tab_s": NS},
        "types": [
            {"name": "bb", "tab": "tab_b", "Wtot": r_bb.shape[2], "groups": g_bb,
             "gb_meta": m_bb, "bcols": [a.shape[2] for a in i_bb]},
            {"name": "sb", "tab": "tab_s", "Wtot": r_sb.shape[2], "groups": g_sb,
             "gb_meta": m_sb, "bcols": [a.shape[2] for a in i_sb]},
            {"name": "bs", "tab": "tab_b", "Wtot": r_bs.shape[2], "groups": g_bs,
             "gb_meta": m_bs, "bcols": [a.shape[2] for a in i_bs]},
        ],
        "head": False, "out_s": True,
    }
    ncA = _build_launch(cfgA)
    wA = _pack_weights(cfgA, Wl[0, 0], Wl[0, 1], Wr[0, 0] + Wr[0, 1],
                       bl[0, 0] + bl[0, 1], bs_bias=bl[0, 2],
                       Wlbs=Wl[0, 2], Wrs=Wr[0, 2])
    def tab_splits(tab):
        return {f"{n}_{b}": np.ascontiguousarray(tab[b * BUCK:(b + 1) * BUCK])
                for b in range((tab.shape[0] + BUCK - 1) // BUCK)
                for n in [None]}

    def bucket_ins(name, arrs, c):
        return {f"{name}_{b}": (a[c] if a.shape[2] >= 16 else
                                np.zeros((P, 16), np.int16))
                for b, a in enumerate(arrs)}

    tb = {f"tab_b_{b}": np.ascontiguousarray(x_b[b * BUCK:(b + 1) * BUCK])
          for b in range(4)}
    tsp = {f"tab_s_{b}": np.ascontiguousarray(x_s[b * BUCK:(b + 1) * BUCK])
           for b in range(2)}
    in_maps = []
    for c in range(NCORES):
        in_maps.append({
            **tb, **tsp,
            "xbT": np.ascontiguousarray(x_b[c::NCORES].T),
            "xsT": np.ascontiguousarray(x_s[c::NCORES].T),
            "wts": wA,
            **bucket_ins("idx_bb", i_bb, c), "rel_bb": r_bb[c], "ivc_bb": v_bb[c],
            **bucket_ins("idx_sb", i_sb, c), "rel_sb": r_sb[c], "ivc_sb": v_sb[c],
            **bucket_ins("idx_bs", i_bs, c), "rel_bs": r_bs[c], "ivc_bs": v_bs[c],
        })
    _tr = False
    _t0 = time.time()
    resA = run_bass_kernel_spmd(ncA, in_maps, core_ids=list(range(NCORES)),
                                trace=_tr, trace_cores=[0] if _tr else None)
    _execA = time.time() - _t0
    if _tr:
        print("launchA exec_ns:", resA.exec_time_ns,
              "trace:", (resA.instructions_and_trace or (None, None))[1], flush=True)
    nbT = [resA.results[c]["nbT"] for c in range(NCORES)]
    nsT = [resA.results[c]["nsT"] for c in range(NCORES)]

    # ---------------- layer 2: host halo exchange + index translation
    xb1 = np.concatenate([t.T for t in nbT], 0)   # [NB, D] core-block order
    xs1 = np.concatenate([t.T for t in nsT], 0)   # [NS, D]

    def tr_b(v):
        return (v % NCORES) * NLB + v // NCORES

    def tr_s(v):
        return (v % NCORES) * NLS + v // NCORES

    pc_bb2 = _shard_edges(tr_b(ei_bb[0]), ei_bb[1], NB)
    pc_sb2 = _shard_edges(tr_s(ei_sb[0]), ei_sb[1], NB)
    i_bb2, r_bb2, v_bb2, g_bb2, m_bb2 = _pack_type(pc_bb2, NLB, NB)
    i_sb2, r_sb2, v_sb2, g_sb2, m_sb2 = _pack_type(pc_sb2, NS // NCORES, NS)

    cfgB = {
        "tabs": {"tab_b": NB, "tab_s": NS},
        "types": [
            {"name": "bb", "tab": "tab_b", "Wtot": r_bb2.shape[2], "groups": g_bb2,
             "gb_meta": m_bb2, "bcols": [a.shape[2] for a in i_bb2]},
            {"name": "sb", "tab": "tab_s", "Wtot": r_sb2.shape[2], "groups": g_sb2,
             "gb_meta": m_sb2, "bcols": [a.shape[2] for a in i_sb2]},
        ],
        "head": True, "out_s": False,
    }
    ncB = _build_launch(cfgB)
    wB = _pack_weights(cfgB, Wl[1, 0], Wl[1, 1], Wr[1, 0] + Wr[1, 1],
                       bl[1, 0] + bl[1, 1], WhT=Wh.T, bh0=bh)
    tb1 = {f"tab_b_{b}": np.ascontiguousarray(xb1[b * BUCK:(b + 1) * BUCK])
           for b in range(4)}
    ts1 = {f"tab_s_{b}": np.ascontiguousarray(xs1[b * BUCK:(b + 1) * BUCK])
           for b in range(2)}
    in_mapsB = []
    for c in range(NCORES):
        in_mapsB.append({
            **tb1, **ts1,
            "xbT": nbT[c], "wts": wB,
            **bucket_ins("idx_bb", i_bb2, c), "rel_bb": r_bb2[c], "ivc_bb": v_bb2[c],
            **bucket_ins("idx_sb", i_sb2, c), "rel_sb": r_sb2[c], "ivc_sb": v_sb2[c],
        })
    _t0 = time.time()
    resB = run_bass_kernel_spmd(ncB, in_mapsB, core_ids=list(range(NCORES)),
                                trace=_tr, trace_cores=[0] if _tr else None)
    _execB = time.time() - _t0
    if _tr:
        print("launchB exec_ns:", resB.exec_time_ns,
              "trace:", (resB.instructions_and_trace or (None, None))[1], flush=True)
    global LAST_HW_NS, LAST_EXEC_S
    if resA.exec_time_ns and resB.exec_time_ns:
        LAST_HW_NS = int(resA.exec_time_ns) + int(resB.exec_time_ns)
    LAST_EXEC_S = (_execA, _execB)

    y = np.empty((NB, 8), np.float32)
    for c in range(NCORES):
        y[np.arange(NLB) * NCORES + c] = resB.results[c]["yT"].T
    return y
